# revision 48
# baseline (speedup 1.0000x reference)
"""Trainium2 Bass kernel for nn_MultiHeadAttention_77232101917088.

Causal MHA where only the LAST token's projected output is returned:
    out = (softmax_causal(q k^T / sqrt(hd)) v)[:, -1, :] @ Wo + bo

Only the last query row survives, so the problem collapses (the last
causal row attends to every position):
    q[b,:]        = x[b,-1,:] @ Wq
    u[b,h,d]      = sum_e Wk[d, h*128+e] * q[b, h*128+e]
    scores[b,h,j] = sum_d x[b,j,d] * u[b,h,d]           (no K/V materialized)
    p             = softmax_j(scores * 1/sqrt(hd))
    w[b,h,d]      = sum_j p[b,h,j] * x[b,j,d]
    ctx[b, hs]    = w[b,h,:] @ Wv[:, hs]
    out           = ctx @ Wo + bo

Sharding: ZERO collectives (first-collective init costs ~74us wall on
this stack).  Each core owns one batch and 4 heads (b = core//4, head
group = core%4), computing its 4 (b,h) pairs end to end from full-depth
x[b] in both layouts; the host sums the 4 output partials per batch.

Schedule: ZERO mid-kernel DMAs — all small transposes (u, scores, w,
ctx) run on the PE in transpose mode, so nothing ever waits on the 8
shared HWDGE completion lanes (DMA bounces were measured stalling
15-35us behind unrelated bulk-load lane reuse).  The 24MB of inputs
stream on two deep rings (scalar HWDGE / gpsimd SWDGE) ordered
[weights, xT half, xn quarters, late weight] so each pipeline stage's
data lands just in time; w accumulates per xn quarter as it arrives.
The softmax z comes from exp-with-accum_out on the scalar engine;
1/sqrt(hd) is folded into exp's scale; max-subtraction is skipped
(|scores*ISCALE| < ~5 for this input class).  All data is bf16.
"""

import numpy as np
from ml_dtypes import bfloat16

import concourse.bacc as bacc
import concourse.bass as bass
import concourse.mybir as mybir
import concourse.tile as tile
from concourse.bass_utils import run_bass_kernel_spmd

P = 128          # partitions
B = 2            # batch
S = 2048         # sequence length
D = 2048         # model dim
NH = 16          # heads
HD = 128         # head dim
NC = 8           # cores
HPC = 4          # heads per core
HW = HPC * HD    # per-core head-column width (512)
DT = D // P      # depth subtiles (16)
JT = S // P      # sequence subtiles (16)
NJC = 4          # 512-wide chunks for streaming matmuls
JC = S // NJC    # 512
HJ = S // 2      # j-half width (1024)
QT = JT // 4     # subtiles per xn quarter (4)
ISCALE = 1.0 / np.sqrt(HD)

FP32 = mybir.dt.float32
BF16 = mybir.dt.bfloat16


def _build_program():
    nc = bacc.Bacc(
        "TRN2",
        target_bir_lowering=False,
        debug=False,
        enable_asserts=False,
        num_devices=NC,
    )

    # ---- per-core DRAM inputs (host pre-arranged, contiguous loads) ------
    xlastT = nc.dram_tensor("xlastT", [P, DT], BF16, kind="ExternalInput").ap()
    ident = nc.dram_tensor("ident", [HPC, HPC], BF16, kind="ExternalInput").ap()
    wq = nc.dram_tensor("wq", [P, DT, HW], BF16, kind="ExternalInput").ap()
    wkT = nc.dram_tensor("wkT", [P, HPC, D], BF16, kind="ExternalInput").ap()
    xTa = nc.dram_tensor("xTa", [P, DT, HJ], BF16, kind="ExternalInput").ap()
    xTb = nc.dram_tensor("xTb", [P, DT, HJ], BF16, kind="ExternalInput").ap()
    xnq = [nc.dram_tensor(f"xnq{i}", [P, QT, D], BF16, kind="ExternalInput").ap()
           for i in range(4)]
    wva = nc.dram_tensor("wva", [P, DT // 2, HW], BF16, kind="ExternalInput").ap()
    wvb = nc.dram_tensor("wvb", [P, DT // 2, HW], BF16, kind="ExternalInput").ap()
    woa = nc.dram_tensor("woa", [P, 2, D], BF16, kind="ExternalInput").ap()
    wob = nc.dram_tensor("wob", [P, 2, D], BF16, kind="ExternalInput").ap()
    bo_sh = nc.dram_tensor("bo_sh", [D], BF16, kind="ExternalInput").ap()

    out_sh = nc.dram_tensor("out_sh", [1, D], FP32, kind="ExternalOutput").ap()

    with tile.TileContext(nc) as tc:
        with (
            tc.tile_pool(name="persist", bufs=1) as pp,
            tc.tile_pool(name="work", bufs=1) as wp,
            tc.tile_pool(name="psA", bufs=2, space="PSUM") as psA,
            tc.tile_pool(name="psW", bufs=1, space="PSUM") as psW,
            tc.tile_pool(name="psB", bufs=2, space="PSUM") as psB,
        ):
            # ---- loads: two deep bulk rings, sync only tiny + out -------
            xlastT_sb = pp.tile([P, DT], BF16, name="xlastT_sb")
            nc.sync.dma_start(xlastT_sb[:], xlastT)
            ident_sb = pp.tile([HPC, HPC], BF16, name="ident_sb")
            nc.sync.dma_start(ident_sb[:], ident)
            # bias rides partition 0 of a zeroed tile; a unit-vector lhsT
            # turns the bias add into one extra matmul accumulation step.
            bo_sb = pp.tile([P, D], BF16, name="bo_sb")
            nc.vector.memset(bo_sb[:], 0.0)
            nc.sync.dma_start(bo_sb[0:1, :], bo_sh.rearrange("(o m) -> o m", o=1))
            e0_sb = pp.tile([P, 1], BF16, name="e0_sb")
            nc.vector.memset(e0_sb[:], 0.0)
            nc.vector.memset(e0_sb[0:1, 0:1], 1.0)

            wq_sb = pp.tile([P, DT, HW], BF16, name="wq_sb")
            nc.scalar.dma_start(wq_sb[:], wq)
            wkT_sb = pp.tile([P, HPC, D], BF16, name="wkT_sb")
            nc.gpsimd.dma_start(wkT_sb[:], wkT)
            xTa_sb = pp.tile([P, DT, HJ], BF16, name="xTa_sb")
            nc.scalar.dma_start(xTa_sb[:], xTa)
            xTb_sb = pp.tile([P, DT, HJ], BF16, name="xTb_sb")
            nc.gpsimd.dma_start(xTb_sb[:], xTb)
            xnq_sb = [pp.tile([P, QT, D], BF16, name=f"xnq_sb{i}")
                      for i in range(4)]
            nc.scalar.dma_start(xnq_sb[0][:], xnq[0])
            nc.gpsimd.dma_start(xnq_sb[1][:], xnq[1])
            nc.scalar.dma_start(xnq_sb[2][:], xnq[2])
            nc.gpsimd.dma_start(xnq_sb[3][:], xnq[3])
            # late weights ride the otherwise-idle sync ring: they finish
            # by ~30us instead of trailing the bulk rows at ~85us, so the
            # ctx/out chain is never weight-gated.  (No mid-kernel DMAs
            # exist anymore, so sync is pure spare bandwidth; its lane
            # predecessors all complete early.)
            wvb_sb = pp.tile([P, DT // 2, HW], BF16, name="wvb_sb")
            nc.sync.dma_start(wvb_sb[:], wvb)
            wob_sb = pp.tile([P, 2, D], BF16, name="wob_sb")
            nc.sync.dma_start(wob_sb[:], wob)
            wva_sb = pp.tile([P, DT // 2, HW], BF16, name="wva_sb", tag="wq_sb")
            nc.sync.dma_start(wva_sb[:], wva)
            woa_sb = pp.tile([P, 2, D], BF16, name="woa_sb", tag="wkT_sb")
            nc.sync.dma_start(woa_sb[:], woa)

            # ---- A: q = xlast @ Wq[:, hs]  ([1, 512]) -------------------
            ps_q = psB.tile([1, HW], FP32, name="ps_q", tag="psB")
            for t in range(DT):
                nc.tensor.matmul(
                    ps_q[:],
                    lhsT=xlastT_sb[:, t:t + 1],
                    rhs=wq_sb[:, t, :],
                    start=(t == 0),
                    stop=(t == DT - 1),
                )
            q_sb = wp.tile([1, HW], BF16, name="q_sb")
            nc.vector.tensor_copy(q_sb[:], ps_q[:])
            qT_sb = wp.tile([P, HPC], BF16, name="qT_sb")
            for es in range(HPC):
                ps_qt = psB.tile([P, 1], BF16, name="ps_qt", tag="psB")
                nc.tensor.transpose(
                    ps_qt[:], q_sb[:, es * P:(es + 1) * P], ident_sb[:1, :1]
                )
                nc.vector.tensor_copy(qT_sb[:, es:es + 1], ps_qt[:])
            qtil_sb = wp.tile([P, HPC, HPC], BF16, name="qtil_sb")
            nc.vector.memset(qtil_sb[:], 0.0)
            for es in range(HPC):
                nc.vector.tensor_copy(
                    qtil_sb[:, es, es:es + 1], qT_sb[:, es:es + 1])

            # ---- B: u[h, d], then PE-transpose to uT[p, t, h] -----------
            u_sb = wp.tile([HPC, D], BF16, name="u_sb", tag="udw")
            for oc in range(NJC):
                ps_u = psB.tile([HPC, JC], FP32, name="ps_u", tag="psB")
                for es in range(HPC):
                    nc.tensor.matmul(
                        ps_u[:],
                        lhsT=qtil_sb[:, es, :],
                        rhs=wkT_sb[:, es, oc * JC:(oc + 1) * JC],
                        start=(es == 0),
                        stop=(es == HPC - 1),
                    )
                nc.vector.tensor_copy(u_sb[:, oc * JC:(oc + 1) * JC], ps_u[:])
            uT_sb = wp.tile([P, DT, HPC], BF16, name="uT_sb")
            for t in range(DT):
                ps_t = psB.tile([P, HPC], BF16, name="ps_ut", tag="psB")
                nc.tensor.transpose(
                    ps_t[:], u_sb[:, t * P:(t + 1) * P], ident_sb[:]
                )
                nc.vector.tensor_copy(uT_sb[:, t, :], ps_t[:])

            # ---- C: scores per j-half; PE-transpose + exp ---------------
            sc_sb = wp.tile([HPC, S], BF16, name="sc_sb", tag="udw")
            eT_sb = wp.tile([P, JT, HPC], BF16, name="eT_sb")
            z2_sb = wp.tile([HPC, 2], FP32, name="z2_sb")
            xT_half = [xTa_sb, xTb_sb]
            for jc in range(NJC):
                ps_s = psA.tile([HPC, JC], FP32, name="ps_s", tag="psA")
                xTh = xT_half[jc // 2]
                base = (jc % 2) * JC
                for t in range(DT):
                    nc.tensor.matmul(
                        ps_s[:],
                        lhsT=uT_sb[:, t, :],
                        rhs=xTh[:, t, base:base + JC],
                        start=(t == 0),
                        stop=(t == DT - 1),
                    )
                nc.vector.tensor_copy(sc_sb[:, jc * JC:(jc + 1) * JC], ps_s[:])
                if jc % 2 == 1:
                    h = jc // 2
                    for lt in range(JT // 2):
                        t = h * (JT // 2) + lt
                        ps_e = psB.tile([P, HPC], BF16, name="ps_e", tag="psB")
                        nc.tensor.transpose(
                            ps_e[:], sc_sb[:, t * P:(t + 1) * P], ident_sb[:]
                        )
                        nc.vector.tensor_copy(eT_sb[:, t, :], ps_e[:])
                    nc.scalar.activation(
                        eT_sb[:, h * (JT // 2):(h + 1) * (JT // 2), :],
                        eT_sb[:, h * (JT // 2):(h + 1) * (JT // 2), :],
                        mybir.ActivationFunctionType.Exp, scale=float(ISCALE),
                    )
                    # softmax denominator for this half via exp+accum
                    # (in-place over sc: the transposes above already read it)
                    nc.scalar.activation(
                        sc_sb[:, h * HJ:(h + 1) * HJ],
                        sc_sb[:, h * HJ:(h + 1) * HJ],
                        mybir.ActivationFunctionType.Exp, scale=float(ISCALE),
                        accum_out=z2_sb[:, h:h + 1],
                    )
            rz_sb = wp.tile([HPC, 1], FP32, name="rz_sb")
            z_sb = wp.tile([HPC, 1], FP32, name="z_sb")
            nc.vector.tensor_tensor(
                z_sb[:], z2_sb[:, 0:1], z2_sb[:, 1:2], mybir.AluOpType.add)
            nc.vector.reciprocal(rz_sb[:], z_sb[:])

            # ---- D: w_un accumulated per xn quarter ---------------------
            w_sb = wp.tile([HPC, D], BF16, name="w_sb", tag="udw")
            ps_w = [psW.tile([HPC, JC], FP32, name=f"ps_w{oc}", tag=f"psW{oc}")
                    for oc in range(NJC)]
            for qi in range(4):
                for oc in range(NJC):
                    for jt in range(QT):
                        nc.tensor.matmul(
                            ps_w[oc][:],
                            lhsT=eT_sb[:, qi * QT + jt, :],
                            rhs=xnq_sb[qi][:, jt, oc * JC:(oc + 1) * JC],
                            start=(qi == 0 and jt == 0),
                            stop=(qi == 3 and jt == QT - 1),
                        )
            for oc in range(NJC):
                if oc % 2 == 0:
                    nc.vector.tensor_scalar_mul(
                        w_sb[:, oc * JC:(oc + 1) * JC], ps_w[oc][:], rz_sb[:])
                else:
                    nc.scalar.activation(
                        w_sb[:, oc * JC:(oc + 1) * JC], ps_w[oc][:],
                        mybir.ActivationFunctionType.Copy, scale=rz_sb[:],
                    )
            wT_sb = wp.tile([P, DT, HPC], BF16, name="wT_sb")
            for t in range(DT):
                ps_t = psB.tile([P, HPC], BF16, name="ps_wt", tag="psB")
                nc.tensor.transpose(
                    ps_t[:], w_sb[:, t * P:(t + 1) * P], ident_sb[:]
                )
                nc.vector.tensor_copy(wT_sb[:, t, :], ps_t[:])

            # ---- E: ctx full [h, c'], keep per-head diagonal blocks -----
            cf_sb = wp.tile([HPC, HW], BF16, name="cf_sb")
            ps_cf = psA.tile([HPC, HW], FP32, name="ps_cf", tag="psA")
            for t in range(DT):
                wvh = wva_sb if t < DT // 2 else wvb_sb
                nc.tensor.matmul(
                    ps_cf[:],
                    lhsT=wT_sb[:, t, :],
                    rhs=wvh[:, t % (DT // 2), :],
                    start=(t == 0),
                    stop=(t == DT - 1),
                )
            nc.vector.tensor_copy(cf_sb[:], ps_cf[:])
            ctxT_sb = wp.tile([P, HPC, 1], BF16, name="ctxT_sb")
            for h in range(HPC):
                ps_ct = psB.tile([P, HPC], BF16, name="ps_ct", tag="psB")
                nc.tensor.transpose(
                    ps_ct[:], cf_sb[:, h * HD:(h + 1) * HD], ident_sb[:]
                )
                nc.vector.tensor_copy(ctxT_sb[:, h, :], ps_ct[:, h:h + 1])

            # ---- F: out partial = ctx_vec @ Wo[hs, :] + bo/4 ------------
            # bias added as a 5th accumulation step (e0 . bo rank-1 matmul)
            o_sb = wp.tile([1, D], FP32, name="o_sb")
            for oc in range(NJC):
                ps_o = psA.tile([1, JC], FP32, name="ps_o", tag="psA")
                for sub in range(HPC):
                    woh = woa_sb if sub < 2 else wob_sb
                    nc.tensor.matmul(
                        ps_o[:],
                        lhsT=ctxT_sb[:, sub, :],
                        rhs=woh[:, sub % 2, oc * JC:(oc + 1) * JC],
                        start=(sub == 0),
                        stop=False,
                    )
                nc.tensor.matmul(
                    ps_o[:],
                    lhsT=e0_sb[:],
                    rhs=bo_sb[:, oc * JC:(oc + 1) * JC],
                    start=False,
                    stop=True,
                )
                eng = nc.vector if oc % 2 == 0 else nc.scalar
                if oc % 2 == 0:
                    eng.tensor_copy(o_sb[:, oc * JC:(oc + 1) * JC], ps_o[:])
                else:
                    eng.activation(
                        o_sb[:, oc * JC:(oc + 1) * JC], ps_o[:],
                        mybir.ActivationFunctionType.Copy,
                    )
            nc.sync.dma_start(out_sh[:], o_sb[:])

    nc.compile()
    return nc


_PROGRAM = None


def _get_program():
    global _PROGRAM
    if _PROGRAM is None:
        _PROGRAM = _build_program()
    return _PROGRAM


def _shard_inputs(x, Wq, Wk, Wv, Wo, bo):
    xb = x.astype(bfloat16)
    wqb = Wq.astype(bfloat16)
    wkb = Wk.astype(bfloat16)
    wvb = Wv.astype(bfloat16)
    wob = Wo.astype(bfloat16)
    bo4 = (bo / HPC).astype(bfloat16)
    identity = np.eye(HPC, dtype=bfloat16)

    in_maps = []
    for core in range(NC):
        b = core // HPC
        hg = core % HPC
        hs = slice(hg * HW, (hg + 1) * HW)
        xlastT_pre = np.ascontiguousarray(xb[b, -1, :].reshape(DT, P).T)
        wq_pre = np.ascontiguousarray(
            wqb[:, hs].reshape(DT, P, HW).transpose(1, 0, 2))
        wkT_pre = np.ascontiguousarray(
            wkb[:, hs].T.reshape(HPC, P, D).transpose(1, 0, 2))
        xT_pre = xb[b].T.reshape(DT, P, S).transpose(1, 0, 2)
        xTa_pre = np.ascontiguousarray(xT_pre[:, :, 0:HJ])
        xTb_pre = np.ascontiguousarray(xT_pre[:, :, HJ:S])
        xn_pre = xb[b].reshape(JT, P, D).transpose(1, 0, 2)
        xnq_pre = [np.ascontiguousarray(xn_pre[:, i * QT:(i + 1) * QT, :])
                   for i in range(4)]
        wv_pre = wvb[:, hs].reshape(DT, P, HW).transpose(1, 0, 2)
        wo_pre = wob[hs, :].reshape(HPC, P, D).transpose(1, 0, 2)
        m = {
            "xlastT": xlastT_pre,
            "ident": identity,
            "wq": wq_pre,
            "wkT": wkT_pre,
            "xTa": xTa_pre,
            "xTb": xTb_pre,
            "wva": np.ascontiguousarray(wv_pre[:, 0:DT // 2, :]),
            "wvb": np.ascontiguousarray(wv_pre[:, DT // 2:DT, :]),
            "woa": np.ascontiguousarray(wo_pre[:, 0:2, :]),
            "wob": np.ascontiguousarray(wo_pre[:, 2:4, :]),
            "bo_sh": bo4,
        }
        for i in range(4):
            m[f"xnq{i}"] = xnq_pre[i]
        in_maps.append(m)
    return in_maps


def kernel(x, Wq, Wk, Wv, Wo, bo, _trace=False, _trace_cores=None):
    x = np.asarray(x, dtype=np.float32)
    Wq = np.asarray(Wq, dtype=np.float32)
    Wk = np.asarray(Wk, dtype=np.float32)
    Wv = np.asarray(Wv, dtype=np.float32)
    Wo = np.asarray(Wo, dtype=np.float32)
    bo = np.asarray(bo, dtype=np.float32)

    nc = _get_program()
    in_maps = _shard_inputs(x, Wq, Wk, Wv, Wo, bo)
    res = run_bass_kernel_spmd(
        nc, in_maps, core_ids=list(range(NC)),
        trace=_trace, trace_cores=_trace_cores,
    )
    out = np.zeros((B, D), dtype=np.float32)
    for core in range(NC):
        out[core // HPC] += res.results[core]["out_sh"][0]
    if _trace:
        kernel._last_results = res
    return out


# revision 49
# speedup vs baseline: 1.1072x; 1.1072x over previous
"""Trainium2 Bass kernel for nn_MultiHeadAttention_77232101917088.

Causal MHA where only the LAST token's projected output is returned:
    out = (softmax_causal(q k^T / sqrt(hd)) v)[:, -1, :] @ Wo + bo

Only the last query row survives, so the problem collapses (the last
causal row attends to every position):
    q[b,:]        = x[b,-1,:] @ Wq
    u[b,h,d]      = sum_e Wk[d, h*128+e] * q[b, h*128+e]
    scores[b,h,j] = sum_d x[b,j,d] * u[b,h,d]           (no K/V materialized)
    p             = softmax_j(scores * 1/sqrt(hd))
    w[b,h,d]      = sum_j p[b,h,j] * x[b,j,d]
    ctx[b, hs]    = w[b,h,:] @ Wv[:, hs]
    out           = ctx @ Wo + bo

Sharding: ZERO collectives (first-collective init costs ~74us wall on
this stack).  Each core owns one batch and 4 heads (b = core//4, head
group = core%4), computing its 4 (b,h) pairs end to end from full-depth
x[b] in both layouts; the host sums the 4 output partials per batch.

Schedule: ZERO mid-kernel DMAs — all small transposes (u, scores, w,
ctx) run on the PE in transpose mode, so nothing ever waits on the 8
shared HWDGE completion lanes (DMA bounces were measured stalling
15-35us behind unrelated bulk-load lane reuse).  The 24MB of inputs
stream on two deep rings (scalar HWDGE / gpsimd SWDGE) ordered
[weights, xT half, xn quarters, late weight] so each pipeline stage's
data lands just in time; w accumulates per xn quarter as it arrives.
The softmax z comes from exp-with-accum_out on the scalar engine;
1/sqrt(hd) is folded into exp's scale; max-subtraction is skipped
(|scores*ISCALE| < ~5 for this input class).  All data is bf16.
"""

import numpy as np
from ml_dtypes import bfloat16

import concourse.bacc as bacc
import concourse.bass as bass
import concourse.mybir as mybir
import concourse.tile as tile
from concourse.bass_utils import run_bass_kernel_spmd

P = 128          # partitions
B = 2            # batch
S = 2048         # sequence length
D = 2048         # model dim
NH = 16          # heads
HD = 128         # head dim
NC = 8           # cores
HPC = 4          # heads per core
HW = HPC * HD    # per-core head-column width (512)
DT = D // P      # depth subtiles (16)
JT = S // P      # sequence subtiles (16)
NJC = 4          # 512-wide chunks for streaming matmuls
JC = S // NJC    # 512
HJ = S // 2      # j-half width (1024)
QT = JT // 4     # subtiles per xn quarter (4)
ISCALE = 1.0 / np.sqrt(HD)

FP32 = mybir.dt.float32
BF16 = mybir.dt.bfloat16


def _build_program():
    nc = bacc.Bacc(
        "TRN2",
        target_bir_lowering=False,
        debug=False,
        enable_asserts=False,
        num_devices=NC,
    )

    # ---- per-core DRAM inputs (host pre-arranged, contiguous loads) ------
    xlastT = nc.dram_tensor("xlastT", [P, DT], BF16, kind="ExternalInput").ap()
    ident = nc.dram_tensor("ident", [HPC, HPC], BF16, kind="ExternalInput").ap()
    wq = nc.dram_tensor("wq", [P, DT, HW], BF16, kind="ExternalInput").ap()
    wkT = nc.dram_tensor("wkT", [P, HPC, D], BF16, kind="ExternalInput").ap()
    xTa = nc.dram_tensor("xTa", [P, DT, HJ], BF16, kind="ExternalInput").ap()
    xTb = nc.dram_tensor("xTb", [P, DT, HJ], BF16, kind="ExternalInput").ap()
    xnq = [nc.dram_tensor(f"xnq{i}", [P, QT, D], BF16, kind="ExternalInput").ap()
           for i in range(4)]
    wva = nc.dram_tensor("wva", [P, DT // 2, HW], BF16, kind="ExternalInput").ap()
    wvb = nc.dram_tensor("wvb", [P, DT // 2, HW], BF16, kind="ExternalInput").ap()
    woa = nc.dram_tensor("woa", [P, 2, D], BF16, kind="ExternalInput").ap()
    wob = nc.dram_tensor("wob", [P, 2, D], BF16, kind="ExternalInput").ap()
    bo_sh = nc.dram_tensor("bo_sh", [D], BF16, kind="ExternalInput").ap()

    out_sh = nc.dram_tensor("out_sh", [1, D], FP32, kind="ExternalOutput").ap()

    with tile.TileContext(nc) as tc:
        with (
            tc.tile_pool(name="persist", bufs=1) as pp,
            tc.tile_pool(name="work", bufs=1) as wp,
            tc.tile_pool(name="psA", bufs=2, space="PSUM") as psA,
            tc.tile_pool(name="psW", bufs=1, space="PSUM") as psW,
            tc.tile_pool(name="psB", bufs=2, space="PSUM") as psB,
        ):
            # ---- loads: two deep bulk rings, sync only tiny + out -------
            xlastT_sb = pp.tile([P, DT], BF16, name="xlastT_sb")
            nc.sync.dma_start(xlastT_sb[:], xlastT)
            ident_sb = pp.tile([HPC, HPC], BF16, name="ident_sb")
            nc.sync.dma_start(ident_sb[:], ident)
            # bias rides partition 0 of a zeroed tile; a unit-vector lhsT
            # turns the bias add into one extra matmul accumulation step.
            bo_sb = pp.tile([P, D], BF16, name="bo_sb")
            nc.vector.memset(bo_sb[:], 0.0)
            nc.sync.dma_start(bo_sb[0:1, :], bo_sh.rearrange("(o m) -> o m", o=1))
            e0_sb = pp.tile([P, 1], BF16, name="e0_sb")
            nc.vector.memset(e0_sb[:], 0.0)
            nc.vector.memset(e0_sb[0:1, 0:1], 1.0)

            wq_sb = pp.tile([P, DT, HW], BF16, name="wq_sb")
            nc.scalar.dma_start(wq_sb[:], wq)
            wkT_sb = pp.tile([P, HPC, D], BF16, name="wkT_sb")
            nc.gpsimd.dma_start(wkT_sb[:], wkT)
            xTa_sb = pp.tile([P, DT, HJ], BF16, name="xTa_sb")
            nc.scalar.dma_start(xTa_sb[:], xTa)
            xTb_sb = pp.tile([P, DT, HJ], BF16, name="xTb_sb")
            nc.gpsimd.dma_start(xTb_sb[:], xTb)
            xnq_sb = [pp.tile([P, QT, D], BF16, name=f"xnq_sb{i}")
                      for i in range(4)]
            nc.scalar.dma_start(xnq_sb[0][:], xnq[0])
            nc.gpsimd.dma_start(xnq_sb[1][:], xnq[1])
            nc.scalar.dma_start(xnq_sb[2][:], xnq[2])
            nc.gpsimd.dma_start(xnq_sb[3][:], xnq[3])
            wva_sb = pp.tile([P, DT // 2, HW], BF16, name="wva_sb", tag="wq_sb")
            nc.scalar.dma_start(wva_sb[:], wva)
            woa_sb = pp.tile([P, 2, D], BF16, name="woa_sb", tag="wkT_sb")
            nc.gpsimd.dma_start(woa_sb[:], woa)
            wvb_sb = pp.tile([P, DT // 2, HW], BF16, name="wvb_sb")
            nc.scalar.dma_start(wvb_sb[:], wvb)
            wob_sb = pp.tile([P, 2, D], BF16, name="wob_sb")
            nc.gpsimd.dma_start(wob_sb[:], wob)

            # ---- A: q = xlast @ Wq[:, hs]  ([1, 512]) -------------------
            ps_q = psB.tile([1, HW], FP32, name="ps_q", tag="psB")
            for t in range(DT):
                nc.tensor.matmul(
                    ps_q[:],
                    lhsT=xlastT_sb[:, t:t + 1],
                    rhs=wq_sb[:, t, :],
                    start=(t == 0),
                    stop=(t == DT - 1),
                )
            q_sb = wp.tile([1, HW], BF16, name="q_sb")
            nc.vector.tensor_copy(q_sb[:], ps_q[:])
            qT_sb = wp.tile([P, HPC], BF16, name="qT_sb")
            for es in range(HPC):
                ps_qt = psB.tile([P, 1], BF16, name="ps_qt", tag="psB")
                nc.tensor.transpose(
                    ps_qt[:], q_sb[:, es * P:(es + 1) * P], ident_sb[:1, :1]
                )
                nc.vector.tensor_copy(qT_sb[:, es:es + 1], ps_qt[:])
            qtil_sb = wp.tile([P, HPC, HPC], BF16, name="qtil_sb")
            nc.vector.memset(qtil_sb[:], 0.0)
            for es in range(HPC):
                nc.vector.tensor_copy(
                    qtil_sb[:, es, es:es + 1], qT_sb[:, es:es + 1])

            # ---- B: u[h, d], then PE-transpose to uT[p, t, h] -----------
            u_sb = wp.tile([HPC, D], BF16, name="u_sb", tag="udw")
            for oc in range(NJC):
                ps_u = psB.tile([HPC, JC], FP32, name="ps_u", tag="psB")
                for es in range(HPC):
                    nc.tensor.matmul(
                        ps_u[:],
                        lhsT=qtil_sb[:, es, :],
                        rhs=wkT_sb[:, es, oc * JC:(oc + 1) * JC],
                        start=(es == 0),
                        stop=(es == HPC - 1),
                    )
                nc.vector.tensor_copy(u_sb[:, oc * JC:(oc + 1) * JC], ps_u[:])
            uT_sb = wp.tile([P, DT, HPC], BF16, name="uT_sb")
            for t in range(DT):
                ps_t = psB.tile([P, HPC], BF16, name="ps_ut", tag="psB")
                nc.tensor.transpose(
                    ps_t[:], u_sb[:, t * P:(t + 1) * P], ident_sb[:]
                )
                nc.vector.tensor_copy(uT_sb[:, t, :], ps_t[:])

            # ---- C: scores per j-half; PE-transpose + exp ---------------
            sc_sb = wp.tile([HPC, S], BF16, name="sc_sb", tag="udw")
            eT_sb = wp.tile([P, JT, HPC], BF16, name="eT_sb")
            z2_sb = wp.tile([HPC, 2], FP32, name="z2_sb")
            xT_half = [xTa_sb, xTb_sb]
            for jc in range(NJC):
                ps_s = psA.tile([HPC, JC], FP32, name="ps_s", tag="psA")
                xTh = xT_half[jc // 2]
                base = (jc % 2) * JC
                for t in range(DT):
                    nc.tensor.matmul(
                        ps_s[:],
                        lhsT=uT_sb[:, t, :],
                        rhs=xTh[:, t, base:base + JC],
                        start=(t == 0),
                        stop=(t == DT - 1),
                    )
                nc.vector.tensor_copy(sc_sb[:, jc * JC:(jc + 1) * JC], ps_s[:])
                if jc % 2 == 1:
                    h = jc // 2
                    for lt in range(JT // 2):
                        t = h * (JT // 2) + lt
                        ps_e = psB.tile([P, HPC], BF16, name="ps_e", tag="psB")
                        nc.tensor.transpose(
                            ps_e[:], sc_sb[:, t * P:(t + 1) * P], ident_sb[:]
                        )
                        nc.vector.tensor_copy(eT_sb[:, t, :], ps_e[:])
                    nc.scalar.activation(
                        eT_sb[:, h * (JT // 2):(h + 1) * (JT // 2), :],
                        eT_sb[:, h * (JT // 2):(h + 1) * (JT // 2), :],
                        mybir.ActivationFunctionType.Exp, scale=float(ISCALE),
                    )
                    # softmax denominator for this half via exp+accum
                    # (in-place over sc: the transposes above already read it)
                    nc.scalar.activation(
                        sc_sb[:, h * HJ:(h + 1) * HJ],
                        sc_sb[:, h * HJ:(h + 1) * HJ],
                        mybir.ActivationFunctionType.Exp, scale=float(ISCALE),
                        accum_out=z2_sb[:, h:h + 1],
                    )
            rz_sb = wp.tile([HPC, 1], FP32, name="rz_sb")
            z_sb = wp.tile([HPC, 1], FP32, name="z_sb")
            nc.vector.tensor_tensor(
                z_sb[:], z2_sb[:, 0:1], z2_sb[:, 1:2], mybir.AluOpType.add)
            nc.vector.reciprocal(rz_sb[:], z_sb[:])

            # ---- D: w_un accumulated per xn quarter ---------------------
            w_sb = wp.tile([HPC, D], BF16, name="w_sb", tag="udw")
            ps_w = [psW.tile([HPC, JC], FP32, name=f"ps_w{oc}", tag=f"psW{oc}")
                    for oc in range(NJC)]
            for qi in range(4):
                for oc in range(NJC):
                    for jt in range(QT):
                        nc.tensor.matmul(
                            ps_w[oc][:],
                            lhsT=eT_sb[:, qi * QT + jt, :],
                            rhs=xnq_sb[qi][:, jt, oc * JC:(oc + 1) * JC],
                            start=(qi == 0 and jt == 0),
                            stop=(qi == 3 and jt == QT - 1),
                        )
            for oc in range(NJC):
                if oc % 2 == 0:
                    nc.vector.tensor_scalar_mul(
                        w_sb[:, oc * JC:(oc + 1) * JC], ps_w[oc][:], rz_sb[:])
                else:
                    nc.scalar.activation(
                        w_sb[:, oc * JC:(oc + 1) * JC], ps_w[oc][:],
                        mybir.ActivationFunctionType.Copy, scale=rz_sb[:],
                    )
            wT_sb = wp.tile([P, DT, HPC], BF16, name="wT_sb")
            for t in range(DT):
                ps_t = psB.tile([P, HPC], BF16, name="ps_wt", tag="psB")
                nc.tensor.transpose(
                    ps_t[:], w_sb[:, t * P:(t + 1) * P], ident_sb[:]
                )
                nc.vector.tensor_copy(wT_sb[:, t, :], ps_t[:])

            # ---- E: ctx full [h, c'], keep per-head diagonal blocks -----
            cf_sb = wp.tile([HPC, HW], BF16, name="cf_sb")
            ps_cf = psA.tile([HPC, HW], FP32, name="ps_cf", tag="psA")
            for t in range(DT):
                wvh = wva_sb if t < DT // 2 else wvb_sb
                nc.tensor.matmul(
                    ps_cf[:],
                    lhsT=wT_sb[:, t, :],
                    rhs=wvh[:, t % (DT // 2), :],
                    start=(t == 0),
                    stop=(t == DT - 1),
                )
            nc.vector.tensor_copy(cf_sb[:], ps_cf[:])
            ctxT_sb = wp.tile([P, HPC, 1], BF16, name="ctxT_sb")
            for h in range(HPC):
                ps_ct = psB.tile([P, HPC], BF16, name="ps_ct", tag="psB")
                nc.tensor.transpose(
                    ps_ct[:], cf_sb[:, h * HD:(h + 1) * HD], ident_sb[:]
                )
                nc.vector.tensor_copy(ctxT_sb[:, h, :], ps_ct[:, h:h + 1])

            # ---- F: out partial = ctx_vec @ Wo[hs, :] + bo/4 ------------
            # bias added as a 5th accumulation step (e0 . bo rank-1 matmul)
            o_sb = wp.tile([1, D], FP32, name="o_sb")
            for oc in range(NJC):
                ps_o = psA.tile([1, JC], FP32, name="ps_o", tag="psA")
                for sub in range(HPC):
                    woh = woa_sb if sub < 2 else wob_sb
                    nc.tensor.matmul(
                        ps_o[:],
                        lhsT=ctxT_sb[:, sub, :],
                        rhs=woh[:, sub % 2, oc * JC:(oc + 1) * JC],
                        start=(sub == 0),
                        stop=False,
                    )
                nc.tensor.matmul(
                    ps_o[:],
                    lhsT=e0_sb[:],
                    rhs=bo_sb[:, oc * JC:(oc + 1) * JC],
                    start=False,
                    stop=True,
                )
                eng = nc.vector if oc % 2 == 0 else nc.scalar
                if oc % 2 == 0:
                    eng.tensor_copy(o_sb[:, oc * JC:(oc + 1) * JC], ps_o[:])
                else:
                    eng.activation(
                        o_sb[:, oc * JC:(oc + 1) * JC], ps_o[:],
                        mybir.ActivationFunctionType.Copy,
                    )
            nc.sync.dma_start(out_sh[:], o_sb[:])

    nc.compile()
    return nc


_PROGRAM = None


def _get_program():
    global _PROGRAM
    if _PROGRAM is None:
        _PROGRAM = _build_program()
    return _PROGRAM


def _shard_inputs(x, Wq, Wk, Wv, Wo, bo):
    xb = x.astype(bfloat16)
    wqb = Wq.astype(bfloat16)
    wkb = Wk.astype(bfloat16)
    wvb = Wv.astype(bfloat16)
    wob = Wo.astype(bfloat16)
    bo4 = (bo / HPC).astype(bfloat16)
    identity = np.eye(HPC, dtype=bfloat16)

    in_maps = []
    for core in range(NC):
        b = core // HPC
        hg = core % HPC
        hs = slice(hg * HW, (hg + 1) * HW)
        xlastT_pre = np.ascontiguousarray(xb[b, -1, :].reshape(DT, P).T)
        wq_pre = np.ascontiguousarray(
            wqb[:, hs].reshape(DT, P, HW).transpose(1, 0, 2))
        wkT_pre = np.ascontiguousarray(
            wkb[:, hs].T.reshape(HPC, P, D).transpose(1, 0, 2))
        xT_pre = xb[b].T.reshape(DT, P, S).transpose(1, 0, 2)
        xTa_pre = np.ascontiguousarray(xT_pre[:, :, 0:HJ])
        xTb_pre = np.ascontiguousarray(xT_pre[:, :, HJ:S])
        xn_pre = xb[b].reshape(JT, P, D).transpose(1, 0, 2)
        xnq_pre = [np.ascontiguousarray(xn_pre[:, i * QT:(i + 1) * QT, :])
                   for i in range(4)]
        wv_pre = wvb[:, hs].reshape(DT, P, HW).transpose(1, 0, 2)
        wo_pre = wob[hs, :].reshape(HPC, P, D).transpose(1, 0, 2)
        m = {
            "xlastT": xlastT_pre,
            "ident": identity,
            "wq": wq_pre,
            "wkT": wkT_pre,
            "xTa": xTa_pre,
            "xTb": xTb_pre,
            "wva": np.ascontiguousarray(wv_pre[:, 0:DT // 2, :]),
            "wvb": np.ascontiguousarray(wv_pre[:, DT // 2:DT, :]),
            "woa": np.ascontiguousarray(wo_pre[:, 0:2, :]),
            "wob": np.ascontiguousarray(wo_pre[:, 2:4, :]),
            "bo_sh": bo4,
        }
        for i in range(4):
            m[f"xnq{i}"] = xnq_pre[i]
        in_maps.append(m)
    return in_maps


def kernel(x, Wq, Wk, Wv, Wo, bo, _trace=False, _trace_cores=None):
    x = np.asarray(x, dtype=np.float32)
    Wq = np.asarray(Wq, dtype=np.float32)
    Wk = np.asarray(Wk, dtype=np.float32)
    Wv = np.asarray(Wv, dtype=np.float32)
    Wo = np.asarray(Wo, dtype=np.float32)
    bo = np.asarray(bo, dtype=np.float32)

    nc = _get_program()
    in_maps = _shard_inputs(x, Wq, Wk, Wv, Wo, bo)
    res = run_bass_kernel_spmd(
        nc, in_maps, core_ids=list(range(NC)),
        trace=_trace, trace_cores=_trace_cores,
    )
    out = np.zeros((B, D), dtype=np.float32)
    for core in range(NC):
        out[core // HPC] += res.results[core]["out_sh"][0]
    if _trace:
        kernel._last_results = res
    return out
